# revision 6
# baseline (speedup 1.0000x reference)
"""MoE-routed DeepQNetwork kernel for 8x Trainium2 NeuronCores.

Problem: B=65536 rows, each routed to one of E=8 expert MLPs
(256 -> 64 -> 64 -> 64 -> 64 -> 64 -> 18, ReLU between layers).

Strategy (expert-grouped sharding, v2):
  Host: stable-sort rows by expert, pad each expert group to a multiple of
  1024 rows so every PAIR of 512-row blocks is single-expert, split the
  sorted+padded batch into 8 equal per-core chunks. Each core runs a static
  expert-agnostic program; expert identity is carried in per-core weight /
  bias tensors (per-pair compact layout, 676 fp16 cols each).

  Device (per core, SPMD): x^T arrives as one [128, 2048] fp16 tile per
  pair (both D-halves of both blocks) so each pair costs ONE dma_start
  (4KB/partition lines; dma_start issue costs ~565ns of sequencer time, so
  fewer+bigger transfers matter more than anything else). Pairs are
  processed in a diagonal pair/layer wavefront so the PE never waits on a
  ReLU of the same pair -> it stays continuously busy and ramps to the
  2.4GHz p-state (idle PE throttles to 1.2/0.65 GHz). L1 runs per-block on
  PE column groups (M=64); L2-5 as [128,128] block-diag; L6 as [128,36]
  block-diag (18+18 rows, no pad). PSUM stays fp32; ReLU+bias run
  PSUM->SBUF on alternating Vector/Scalar engines; the first/last pairs
  split each ReLU across both engines to shorten the pipeline fill/drain.
  L6 results for two pairs accumulate in one [36,1024] PSUM tile, move to
  SBUF with a single copy, and DMA out (b6 is added on the host).

  DMA issue plan: sync(SP) issues the 9 x tiles then the 5 y outputs;
  gpsimd(Pool) issues weights (3 chunks, pair-0's first) and bias.

  Host: unsort the [36, rows] outputs back to original row order, + b6.
"""

import math
import os

import numpy as np

E = 8
D = 256
H = 64
A = 18
NCORES = 8
BLK = 512  # rows per block (matmul moving-operand free dim)
PAIR = 2 * BLK  # rows per pair; expert groups padded to this granularity

# per-pair fp16 weight tensor column layout (WCOLS = 676):
#   [0:64)    W1 K-half0  [128, 64]
#   [64:128)  W1 K-half1  [128, 64]
#   [128:640) W2..W5, each [128, 128] block-diag: [0:64, 0:64] = W_l[e],
#             [64:128, 64:128] = W_l[e]
#   [640:676) W6 [128, 36] block-diag: [0:64, 0:18] = W6[e],
#             [64:128, 18:36] = W6[e]
WCOLS = 676

_PROGRAM_CACHE: dict = {}
LAST_RESULTS = None  # test harness can read timing/profile info from here


def _build_program(npair: int):
    """Build the SPMD bass program for npair single-expert 1024-row pairs."""
    import concourse.mybir as mybir
    import concourse.tile as tile
    from concourse import bacc

    f32 = mybir.dt.float32
    f16 = mybir.dt.float16
    Relu = mybir.ActivationFunctionType.Relu
    add = mybir.AluOpType.add
    amax = mybir.AluOpType.max

    ngrp = (npair + 1) // 2  # output groups of 2 pairs

    nc = bacc.Bacc("TRN2")
    xall = nc.declare_dram_parameter("xall", [128, npair * 2048], f16, isOutput=False)
    wall = nc.declare_dram_parameter("wall", [128, npair * WCOLS], f16, isOutput=False)
    # per pair: cols 0:5 = b1..b5 (rows 0:64 = rows 64:128 = bias of the
    # pair's expert); b6 is added on the host.
    bias = nc.declare_dram_parameter("bias", [128, npair * 5], f32, isOutput=False)
    yt = nc.declare_dram_parameter("yt", [36, npair * BLK], f16, isOutput=True)

    # weight DMA chunks (pair ranges): pair 0 alone so L1(0) starts early,
    # then 2 pairs per chunk so arrival stays ahead of the PE's consumption
    wchunks = [(0, 1)] + [(p, min(p + 2, npair)) for p in range(1, npair, 2)]

    with tile.TileContext(nc) as tc:
        with (
            tc.tile_pool(name="wpool", bufs=1) as wpool,
            tc.tile_pool(name="xpool", bufs=npair) as xpool,
            tc.tile_pool(name="hpool", bufs=3) as hpool,
            tc.tile_pool(name="opool", bufs=3) as opool,
            tc.tile_pool(name="ppool", bufs=4, space="PSUM") as ppool,
            tc.tile_pool(name="popool", bufs=2, space="PSUM") as popool,
        ):
            # ---- PE warm-up: the Tensor engine p-state ramps to 2.4GHz only
            # after ~3us of continuous busy.  Burn the dead time before the
            # first x/w tiles arrive on dummy matmuls over a memset scratch
            # tile so the real stream starts at (near) full clock.
            scratch = wpool.tile([128, BLK], f16, tag="scr", name="scratch", bufs=1)
            nc.gpsimd.memset(scratch[:, :], 0.0)
            for d in range(5):
                pd = ppool.tile([128, BLK], f32, tag="ph", name=f"pdummy_{d}")
                nc.tensor.matmul(
                    out=pd[0:64, :],
                    lhsT=scratch[:, 0:64],
                    rhs=scratch[:, :],
                    start=True,
                    stop=True,
                )

            # ---- input DMAs.  gpsimd: weights + bias; x pair tiles spread
            # over the sync/vector/scalar sequencers (each dma_start costs
            # ~600ns of issue time on its sequencer, so one ring serializes).
            wtiles = {}
            for ci, (p0, p1) in enumerate(wchunks):
                w_c = wpool.tile(
                    [128, (p1 - p0) * WCOLS], f16, tag=f"w{ci}", name=f"w_{ci}", bufs=1
                )
                nc.gpsimd.dma_start(
                    out=w_c[:, :], in_=wall[:, p0 * WCOLS : p1 * WCOLS]
                )
                for p in range(p0, p1):
                    wtiles[p] = (w_c, (p - p0) * WCOLS)
                if ci == 0:
                    bias_sb = wpool.tile(
                        [128, npair * 5], f32, name="bias_sb", tag="bias", bufs=1
                    )
                    nc.gpsimd.dma_start(out=bias_sb[:, :], in_=bias[:, :])

            xts = []
            for p in range(npair):
                xt_p = xpool.tile([128, 2048], f16, tag="x", name=f"x_{p}")
                xeng = (nc.sync, nc.scalar)[p % 2]
                xeng.dma_start(
                    out=xt_p[:, :], in_=xall[:, p * 2048 : (p + 1) * 2048]
                )
                xts.append(xt_p)

            # ---- diagonal pair/layer wavefront --------------------------
            # step s emits layer (s - p) of pair p, newest pair first, so the
            # PE always has >= 1us of independent work between a pair's
            # matmul and the ReLU it depends on.
            hcur = [None] * npair  # current hidden tile per pair
            po_g = [None] * ngrp  # psum output tile per 2-pair group
            o_done = [0] * ngrp  # pairs finished in group

            def relu_out(p, li, ph):
                """bias+ReLU PSUM->SBUF; split across both engines for the
                pipeline-fill/drain pairs, alternate engines otherwise."""
                h = hpool.tile([128, BLK], f16, tag=f"h{li}", name=f"h{li}_{p}")
                w, wo = wtiles[p]
                bap = bias_sb[:, 5 * p + li : 5 * p + li + 1]
                edge = p < 2 or p >= npair - 2
                if edge:
                    hh = BLK // 2
                    nc.vector.tensor_scalar(
                        h[:, 0:hh], ph[:, 0:hh], bap, 0.0, op0=add, op1=amax
                    )
                    nc.scalar.activation(h[:, hh:BLK], ph[:, hh:BLK], Relu, bias=bap)
                elif (p + li) % 2 == 0:
                    nc.vector.tensor_scalar(
                        h[:, :], ph[:, :], bap, 0.0, op0=add, op1=amax
                    )
                else:
                    nc.scalar.activation(h[:, :], ph[:, :], Relu, bias=bap)
                return h

            for s in range(npair + 5):
                for p in range(min(s, npair - 1), max(0, s - 5) - 1, -1):
                    li = s - p
                    w, wo = wtiles[p]
                    if li == 0:
                        # L1: [256 -> 64] per block, blocks on PE col groups
                        ph1 = ppool.tile([128, BLK], f32, tag="ph", name=f"ph1_{p}")
                        xt_p = xts[p]
                        for blk, colr in ((0, slice(0, 64)), (1, slice(64, 128))):
                            for c in range(2):
                                nc.tensor.matmul(
                                    out=ph1[colr, :],
                                    lhsT=w[:, wo + c * 64 : wo + (c + 1) * 64],
                                    rhs=xt_p[
                                        :,
                                        (2 * blk + c) * BLK : (2 * blk + c + 1) * BLK,
                                    ],
                                    start=(c == 0),
                                    stop=(c == 1),
                                )
                        hcur[p] = relu_out(p, 0, ph1)
                    elif li <= 4:
                        # L2-5: [64 -> 64] block-diag [128,128]
                        ph = ppool.tile([128, BLK], f32, tag="ph", name=f"ph{li}_{p}")
                        wc = wo + 128 * li
                        nc.tensor.matmul(
                            out=ph[:, :],
                            lhsT=w[:, wc : wc + 128],
                            rhs=hcur[p][:, :],
                            start=True,
                            stop=True,
                        )
                        hcur[p] = relu_out(p, li, ph)
                    else:
                        # L6: [64 -> 18+18] block-diag, 2 pairs share a
                        # [36, 1024] psum tile (one copy + one DMA per group)
                        g = p // 2
                        gw = 1024 if 2 * g + 1 < npair else 512
                        if po_g[g] is None:
                            po_g[g] = popool.tile(
                                [36, gw], f32, tag="po", name=f"po_{g}"
                            )
                        po = po_g[g]
                        off = (p - 2 * g) * BLK
                        nc.tensor.matmul(
                            out=po[:, off : off + BLK],
                            lhsT=w[:, wo + 640 : wo + 676],
                            rhs=hcur[p][:, :],
                            start=True,
                            stop=True,
                        )
                        o_done[g] += 1
                        if o_done[g] == (2 if gw == 1024 else 1):
                            o_g = opool.tile([36, gw], f16, tag="o", name=f"o_{g}")
                            if g % 2 == 0:
                                nc.vector.tensor_scalar(
                                    o_g[:, :], po[:, :], 0.0, None, op0=add
                                )
                            else:
                                nc.scalar.copy(o_g[:, :], po[:, :])
                            nc.sync.dma_start(
                                out=yt[:, 2 * g * BLK : 2 * g * BLK + gw],
                                in_=o_g[:, :],
                            )

    nc.compile()
    return nc


def _get_program(npair: int):
    if npair not in _PROGRAM_CACHE:
        _PROGRAM_CACHE[npair] = _build_program(npair)
    return _PROGRAM_CACHE[npair]


def _prepare(state, rm_state, W1, b1, W2, b2, W3, b3, W4, b4, W5, b5, W6, b6):
    state = np.ascontiguousarray(np.asarray(state, dtype=np.float32))
    rm = np.asarray(rm_state).reshape(-1).astype(np.int64)
    Ws = [np.asarray(w, dtype=np.float32) for w in (W1, W2, W3, W4, W5, W6)]
    bs = [np.asarray(b, dtype=np.float32) for b in (b1, b2, b3, b4, b5, b6)]
    B = state.shape[0]
    X = state.reshape(B, D)

    # ---- host-side routing: stable sort rows by expert, pad groups so each
    # 1024-row pair is single-expert
    order = np.argsort(rm, kind="stable")
    counts = np.bincount(rm, minlength=E)
    caps = ((counts + PAIR - 1) // PAIR) * PAIR
    caps = np.maximum(caps, PAIR)  # empty groups still occupy one (zero) pair
    T0 = int(caps.sum())
    C = math.ceil(T0 / NCORES / PAIR) * PAIR
    T = NCORES * C
    caps[E - 1] += T - T0  # extend last group's padding to fill all cores
    base = np.zeros(E, dtype=np.int64)
    base[1:] = np.cumsum(caps)[:-1]
    csum = np.zeros(E, dtype=np.int64)
    csum[1:] = np.cumsum(counts)[:-1]
    sorted_expert = rm[order]
    pos_sorted = base[sorted_expert] + (np.arange(B) - csum[sorted_expert])

    Xp = np.zeros((T, D), np.float16)
    Xp[pos_sorted] = X[order].astype(np.float16)
    pair_expert = np.zeros(T // PAIR, np.int64)
    for e in range(E):
        pair_expert[base[e] // PAIR : (base[e] + caps[e]) // PAIR] = e

    W16 = [w.astype(np.float16) for w in Ws]
    npair = C // PAIR

    # per-expert compact weight/bias panels, copied per pair below
    wex = np.zeros((E, 128, WCOLS), np.float16)
    bex = np.zeros((E, 128, 5), np.float32)
    for e in range(E):
        wex[e, :, 0:64] = W16[0][e, 0:128, :]
        wex[e, :, 64:128] = W16[0][e, 128:256, :]
        for li in range(4):
            wc = 128 + li * 128
            wex[e, 0:64, wc : wc + H] = W16[li + 1][e]
            wex[e, 64:128, wc + H : wc + 128] = W16[li + 1][e]
        wex[e, 0:64, 640 : 640 + A] = W16[5][e]
        wex[e, 64:128, 640 + A : 640 + 2 * A] = W16[5][e]
        for li in range(5):
            bex[e, 0:64, li] = bs[li][e]
            bex[e, 64:128, li] = bs[li][e]

    in_maps = []
    for core in range(NCORES):
        xt = Xp[core * C : (core + 1) * C].T  # [D, C] fp16 view
        pe = pair_expert[core * npair : (core + 1) * npair]

        xa = np.empty((128, npair * 2048), np.float16)
        for p in range(npair):
            for blk in range(2):
                src = xt[:, (2 * p + blk) * BLK : (2 * p + blk + 1) * BLK]
                dst = p * 2048 + blk * 1024
                xa[:, dst : dst + BLK] = src[0:128]
                xa[:, dst + BLK : dst + 2 * BLK] = src[128:256]

        wh = wex[pe].transpose(1, 0, 2).reshape(128, npair * WCOLS)
        bh = bex[pe].transpose(1, 0, 2).reshape(128, npair * 5)

        in_maps.append(
            {
                "xall": np.ascontiguousarray(xa),
                "wall": np.ascontiguousarray(wh),
                "bias": np.ascontiguousarray(bh),
            }
        )

    meta = dict(
        B=B, C=C, T=T, npair=npair, order=order, pos_sorted=pos_sorted,
        b6=bs[5], rm=rm,
    )
    return in_maps, meta


def _finalize(results, meta):
    """results: list (per core) of dicts with 'yt' [36, npair*BLK] arrays."""
    B, C, T, npair = (meta[k] for k in ("B", "C", "T", "npair"))
    Yp = np.zeros((T, A), np.float32)
    for core in range(NCORES):
        ytc = results[core]["yt"]
        for p in range(npair):
            cols = slice(p * BLK, (p + 1) * BLK)
            dst = core * C + 2 * p * BLK
            Yp[dst : dst + BLK] = ytc[0:A, cols].T
            Yp[dst + BLK : dst + 2 * BLK] = ytc[A : 2 * A, cols].T

    y = np.zeros((B, A), np.float32)
    y[meta["order"]] = Yp[meta["pos_sorted"]]
    y += meta["b6"][meta["rm"]]
    return y


def kernel(state, rm_state, W1, b1, W2, b2, W3, b3, W4, b4, W5, b5, W6, b6):
    global LAST_RESULTS
    from concourse.bass_utils import run_bass_kernel_spmd

    in_maps, meta = _prepare(
        state, rm_state, W1, b1, W2, b2, W3, b3, W4, b4, W5, b5, W6, b6
    )
    nc = _get_program(meta["npair"])
    trace = bool(os.environ.get("KERNEL_TRACE"))
    res = run_bass_kernel_spmd(nc, in_maps, core_ids=list(range(NCORES)), trace=trace)
    LAST_RESULTS = res
    return _finalize(res.results, meta)
